# revision 57
# baseline (speedup 1.0000x reference)
"""Multi-head self-attention (B=2, N=2048, C=1024, H=16, D=64) on 8 trn2 cores.

Sharding: core c handles batch b = c//4 and the 4 heads [4*(c%4), 4*(c%4)+4).
The host pre-transposes x and the weight slices; per-core partial outputs are
summed on the host, which also adds the output bias. x, the weights, and y
travel as bf16 (the PE computes in bf16/fp32r with fp32 accumulation), which
halves the serialized DMA traffic; measured end-to-end error ~6e-3 absmax
against the fp32 reference (gate 2e-2).

The device program is one software-pipelined stream tuned so the PE is near
gapless and the ScalarE exp stream (the secondary bottleneck: 128 exps of
[128,1024], ~1.04us each) never starves late in the kernel:

  phase A   An ordered single-queue DMA chain delivers the first weight/x
            slices in consumer order (first matmul ~6us in). The four x
            chunks stream through the qkv projection groups, with the
            (pr=0, itl=0) attention block's st->exp->pv chains woven
            between them so the exp stream starts ~13us in. The q tiles
            for itl 2,3 are deferred to phase B.

  phase B   The remaining 7 blocks (pr, itl) run as one global pipeline:
            the score matmul (st) leads its exp by one slot, pv lags the
            exp stream by two slots, and each block's normalization slots
            in right after its last pv. The out-projection groups of query
            tile itl-1 (and the deferred q-projection groups) ride in the
            PE slack of the exp-paced slots. The softmax denominator rides
            as a ones-column inside the PV matmul (PSUM row 64);
            normalization = approximate reciprocal (DVE) + partition
            broadcast (GpSimd) + multiply (DVE). y half-tiles DMA out as
            soon as they evict.

  tail      The last query tile's out-projection rolls its pr=0-dependent
            halves ahead of the final norm; the final norm's PSUM reads run
            on the then-idle ScalarE.

PSUM budget (8 banks): st double-buffer 4, two O~ accumulators 2, matmul
scratch 2."""

import ml_dtypes
import numpy as np

import concourse.bass as bass
import concourse.mybir as mybir
import concourse.tile as tile
from concourse import bacc
from concourse.bass_utils import run_bass_kernel_spmd

F32 = mybir.dt.float32
F32R = mybir.dt.float32r   # TF32-like: 11 mantissa bits, 4x faster PE streaming
BF16 = mybir.dt.bfloat16

B, N, C = 2, 2048, 1024
H, D = 16, 64
HPC = 4            # heads per core
P = 128
FD = 512           # matmul free-dim tile
KB = C // P        # 8 contraction blocks for the projections
NT = N // FD       # 4 free tiles over the sequence
NJB = N // P       # 16 j blocks in attention


class _Ctx:
    """Holds the per-build tile handles shared by the emit helpers."""
    pass


def _emit_qk_group(nc, cx, nt, mt):
    """One qkT projection tile: 8 contraction matmuls + DVE eviction."""
    pq = cx.ps.tile([P, FD], F32, tag="mm", bufs=2, name="pq")
    for kb in range(KB):
        nc.tensor.matmul(
            pq,
            lhsT=cx.wq_sb[:, mt, kb, :],
            rhs=cx.xc[:, kb, :],
            start=(kb == 0),
            stop=(kb == KB - 1),
        )
    nc.vector.tensor_copy(cx.qkT_sb[:, mt, nt * FD:(nt + 1) * FD], pq)


def _emit_v_group(nc, cx, nt, i4):
    """One V projection tile: 8 contraction matmuls + DVE eviction."""
    it = nt * 4 + i4
    pv = cx.ps.tile([P, HPC * D], F32, tag="mm", bufs=2, name="pq")
    for kb in range(KB):
        nc.tensor.matmul(
            pv,
            lhsT=cx.xc[:, kb, i4 * P:(i4 + 1) * P],
            rhs=cx.wq_sb[:, 4:6, kb, :],
            start=(kb == 0),
            stop=(kb == KB - 1),
        )
    nc.vector.tensor_copy(
        cx.V_sb[:, it, :, 0:D], pv.rearrange("p (h d) -> p h d", d=D))


def _emit_st(nc, cx, pr, itl, jb):
    """Score pair (2 heads) for one jb."""
    st = cx.ps.tile([P, 2 * FD], F32, tag="st", bufs=cx.st_bufs, name="st")
    for hh in range(2):
        lo = hh * D
        nc.tensor.matmul(
            st[:, hh * FD:(hh + 1) * FD],
            lhsT=cx.qkT_sb[lo:lo + D, 2 + pr, jb * P:(jb + 1) * P],
            rhs=cx.qkT_sb[lo:lo + D, pr, itl * FD:(itl + 1) * FD],
            start=True, stop=True)
    return st


def _emit_exp(nc, cx, st):
    pt = cx.pt_pool.tile([P, 2 * FD], F32R, tag="pt", name="pt")
    nc.scalar.activation(pt, st, mybir.ActivationFunctionType.Exp, scale=0.125)
    return pt


def _emit_st_exp(nc, cx, pr, itl, jb):
    return _emit_exp(nc, cx, _emit_st(nc, cx, pr, itl, jb))


def _emit_pv(nc, cx, ot, pr, jb, pt):
    """Accumulate P@V (and the ones-row denominator) for both heads."""
    for hh in range(2):
        nc.tensor.matmul(
            ot[hh],
            lhsT=cx.V_sb[:, jb, 2 * pr + hh, :],
            rhs=pt[:, hh * FD:(hh + 1) * FD],
            start=(jb == 0), stop=(jb == NJB - 1))


def _alloc_ot(cx):
    return [cx.ps.tile([D + 1, FD], F32, tag=f"ot{hh}", bufs=1,
                       name=f"ot{hh}") for hh in range(2)]


def _emit_norm(nc, cx, ot, pr, itl, use_act=False):
    """Normalize the accumulated O~ by the softmax denominator.

    The four PSUM reads come first so the ot banks free up for the next
    block's pv accumulation as early as possible. ``use_act`` routes them
    to ScalarE (for the final block, when the exp stream has drained)."""
    cpy = nc.scalar.copy if use_act else nc.vector.tensor_copy
    osls, rins = [], []
    for hh in range(2):
        rin = cx.nrm_pool.tile([1, FD], F32, tag=f"rin{hh}", name=f"rin{hh}")
        cpy(rin, ot[hh][D:D + 1, :])
        osl = cx.oT_sb[:, 2 * pr + hh, itl * FD:(itl + 1) * FD]
        cpy(osl, ot[hh][0:D, :])
        osls.append(osl)
        rins.append(rin)
    for hh in range(2):
        rec = cx.nrm_pool.tile([1, FD], F32, tag="rec", name="rec")
        scr = cx.nrm_pool.tile([1, FD], F32, tag="scr", name="scr")
        nc.vector.reciprocal_approx_accurate(out=rec, in_=rins[hh], scratch=scr)
        rep = cx.nrm_pool.tile([D, FD], F32, tag="rep", name="rep")
        nc.gpsimd.partition_broadcast(rep, rec, channels=D)
        nc.vector.tensor_mul(out=osls[hh], in0=osls[hh], in1=rep)


def _emit_oproj_half(nc, cx, itl, gidx, half):
    """Half of an out-projection group (it, o2): 2 accumulation matmuls;
    the second half also evicts and DMAs the y half-tile."""
    it = itl * 4 + (gidx // 2)
    o2 = gidx % 2
    if half == 0:
        if o2 == 0:
            cx.yts[gidx // 2] = cx.y_pool.tile([P, C], BF16, tag="yt",
                                               bufs=6, name="yt")
        cx.pys[gidx] = cx.ps.tile([P, FD], F32, tag="mm", bufs=2, name="pq")
    py = cx.pys[gidx]
    y_t = cx.yts[gidx // 2]
    for g in (0, 1) if half == 0 else (2, 3):
        nc.tensor.matmul(
            py,
            lhsT=cx.oT_sb[:, g, it * P:(it + 1) * P],
            rhs=cx.wo_sb[:, g, o2 * FD:(o2 + 1) * FD],
            start=(g == 0),
            stop=(g == HPC - 1),
        )
    if half == 1:
        nc.vector.tensor_copy(y_t[:, o2 * FD:(o2 + 1) * FD], py)
        nc.sync.dma_start(cx.y[it * P:(it + 1) * P, o2 * FD:(o2 + 1) * FD],
                           y_t[:, o2 * FD:(o2 + 1) * FD])


def _emit_oproj_group(nc, cx, itl, gidx):
    _emit_oproj_half(nc, cx, itl, gidx, 0)
    _emit_oproj_half(nc, cx, itl, gidx, 1)


def build_nc(repeat: int = 1) -> bacc.Bacc:
    nc = bacc.Bacc("TRN2", target_bir_lowering=False, debug=False)

    xT = nc.dram_tensor("xT", [C, N], BF16, kind="ExternalInput").ap()
    wqkvT = nc.dram_tensor("wqkvT", [P, 6, KB, P], BF16, kind="ExternalInput").ap()
    woutT = nc.dram_tensor("woutT", [HPC, D, C], BF16, kind="ExternalInput").ap()
    vones = nc.dram_tensor("vones", [P, NJB * HPC], F32, kind="ExternalInput").ap()
    y = nc.dram_tensor("y", [N, C], BF16, kind="ExternalOutput").ap()

    xT_r = xT.rearrange("(o p) n -> p o n", p=P)          # [128, 8, 2048]
    woutT_r = woutT.rearrange("g p o -> p g o")           # [64, 4, 1024]

    with tile.TileContext(nc) as tc:
        with (
            tc.tile_pool(name="w_pool", bufs=1) as w_pool,
            tc.tile_pool(name="qk_pool", bufs=1) as qk_pool,
            tc.tile_pool(name="v_pool", bufs=1) as v_pool,
            tc.tile_pool(name="o_pool", bufs=1) as o_pool,
            tc.tile_pool(name="x_pool", bufs=2) as x_pool,
            tc.tile_pool(name="pt_pool", bufs=6) as pt_pool,
            tc.tile_pool(name="y_pool", bufs=6) as y_pool,
            tc.tile_pool(name="nrm_pool", bufs=3) as nrm_pool,
            tc.tile_pool(name="ps", bufs=1, space="PSUM") as ps,
        ):
            cx = _Ctx()
            cx.pys = {}
            cx.yts = {}
            cx.ps = ps
            cx.pt_pool = pt_pool
            cx.y_pool = y_pool
            cx.nrm_pool = nrm_pool
            cx.y = y

            cx.wq_sb = w_pool.tile([P, 6, KB, P], BF16, name="wq_sb")
            cx.wo_sb = w_pool.tile([D, HPC, C], BF16, name="wo_sb")

            cx.qkT_sb = qk_pool.tile([P, 4, N], F32R, name="qkT_sb")
            cx.V_sb = v_pool.tile([P, NJB, HPC, D + 1], F32R, name="V_sb")
            cx.oT_sb = o_pool.tile([D, HPC, N], BF16, name="oT_sb")

            for _rep in range(repeat):
                # ---- phase A: projections + hoisted block (pr=0, itl=0) ----
                # nt2/nt3 q-projection groups are deferred into phase B's
                # first block (which otherwise has no filler PE work).
                cx.st_bufs = 2
                otA = _alloc_ot(cx)
                ptA = [None] * NJB
                deferred = []
                for nt in range(NT):
                    xc = x_pool.tile([P, KB, FD], BF16, tag="xc", name="xc")
                    cx.xc = xc
                    if nt == 0 and _rep == 0:
                        # ordered startup chain: each transfer lands just
                        # before its consumer group needs it
                        nc.sync.dma_start(cx.wq_sb[:, 2], wqkvT[:, 2])
                        nc.sync.dma_start(xc[:, 0:2, :],
                                          xT_r[:, 0:2, 0:FD])
                        nc.sync.dma_start(xc[:, 2:4, :],
                                          xT_r[:, 2:4, 0:FD])
                        nc.sync.dma_start(cx.wq_sb[:, 0], wqkvT[:, 0])
                        nc.sync.dma_start(xc[:, 4:6, :],
                                          xT_r[:, 4:6, 0:FD])
                        nc.sync.dma_start(xc[:, 6:8, :],
                                          xT_r[:, 6:8, 0:FD])
                        nc.sync.dma_start(cx.wq_sb[:, 3], wqkvT[:, 3])
                        nc.sync.dma_start(cx.wq_sb[:, 1], wqkvT[:, 1])
                        nc.sync.dma_start(cx.wq_sb[:, 4], wqkvT[:, 4])
                        nc.sync.dma_start(cx.wq_sb[:, 5], wqkvT[:, 5])
                        nc.sync.dma_start(
                            cx.V_sb[:, :, :, D:D + 1].rearrange(
                                "p j h one -> p j (h one)"),
                            vones.rearrange(
                                "p (j h) -> p j h", h=HPC).bitcast(F32R))
                    else:
                        if nt in (1, 2) and _rep == 0:
                            for g in (0, 1) if nt == 1 else (2, 3):
                                nc.scalar.dma_start(cx.wo_sb[:, g],
                                                    woutT_r[:, g])
                        nc.sync.dma_start(
                            xc[:, 0:KB // 2, :],
                            xT_r[:, 0:KB // 2,
                                 nt * FD:(nt + 1) * FD])
                        nc.gpsimd.dma_start(
                            xc[:, KB // 2:KB, :],
                            xT_r[:, KB // 2:KB,
                                 nt * FD:(nt + 1) * FD])
                    jb0 = 4 * nt
                    # weave: one projection group between each st->exp and
                    # its pv consumer so the exp latency is always covered
                    _emit_qk_group(nc, cx, nt, 2)
                    if nt == 0:
                        _emit_qk_group(nc, cx, nt, 0)
                    ptA[jb0] = _emit_st_exp(nc, cx, 0, 0, jb0)
                    _emit_qk_group(nc, cx, nt, 3)
                    if nt == 0:
                        _emit_qk_group(nc, cx, nt, 1)
                    ptA[jb0 + 1] = _emit_st_exp(nc, cx, 0, 0, jb0 + 1)
                    _emit_v_group(nc, cx, nt, 0)
                    _emit_pv(nc, cx, otA, 0, jb0, ptA[jb0])
                    _emit_v_group(nc, cx, nt, 1)
                    _emit_pv(nc, cx, otA, 0, jb0 + 1, ptA[jb0 + 1])
                    ptA[jb0 + 2] = _emit_st_exp(nc, cx, 0, 0, jb0 + 2)
                    _emit_v_group(nc, cx, nt, 2)
                    _emit_pv(nc, cx, otA, 0, jb0 + 2, ptA[jb0 + 2])
                    ptA[jb0 + 3] = _emit_st_exp(nc, cx, 0, 0, jb0 + 3)
                    _emit_v_group(nc, cx, nt, 3)
                    if nt == 1:
                        _emit_qk_group(nc, cx, nt, 0)
                        _emit_qk_group(nc, cx, nt, 1)
                    elif nt >= 2:
                        deferred.append((xc, nt, 0))
                        deferred.append((xc, nt, 1))
                    _emit_pv(nc, cx, otA, 0, jb0 + 3, ptA[jb0 + 3])

                # ---- phase B: global pipeline over the 7 other blocks ----
                # st leads its exp by one slot; pv lags 2; norms slot in
                # right after a block's last pv; out-proj groups (and the
                # deferred q projections) ride in the PE slack.
                blocks = [(1, 0)] + [(pr, itl) for itl in range(1, NT)
                                     for pr in range(2)]
                NB = len(blocks)
                LAG = 2
                ots = [None] * NB
                pts = [None] * (NB * NJB)

                def _carry(nc, cx, k, jb):
                    # filler PE work for block k at jb slots 3,7,11,15
                    gi = jb // 4
                    if k == 0:
                        xcd, ntd, mtd = deferred[gi]
                        cx.xc = xcd
                        _emit_qk_group(nc, cx, ntd, mtd)
                    else:
                        itl_src = (k - 1) // 2
                        gidx = 4 * ((k - 1) % 2) + gi
                        _emit_oproj_group(nc, cx, itl_src, gidx)

                def _pv_slot(t):
                    # first two pvs of a block wait one extra slot so the
                    # previous block's norm copies can release the ot banks
                    return t + LAG

                pv_next = 0
                sts = [None] * (NB * NJB)
                sts[0] = _emit_st(nc, cx, *blocks[0], 0)
                for s in range(NB * NJB + LAG + 1):
                    if s < NB * NJB:
                        k, jb = divmod(s, NJB)
                        if jb == 0:
                            ots[k] = _alloc_ot(cx)
                        if s + 1 < NB * NJB:
                            k1, jb1 = divmod(s + 1, NJB)
                            sts[s + 1] = _emit_st(nc, cx, *blocks[k1], jb1)
                        pts[s] = _emit_exp(nc, cx, sts[s])
                        if jb % 4 == 3:
                            _carry(nc, cx, k, jb)
                    if s == 0:
                        # hoisted block's norm must precede block 0's first
                        # pv (same psum tags); its st/exp pair covers the
                        # latency
                        _emit_norm(nc, cx, otA, 0, 0)
                    while pv_next < NB * NJB and _pv_slot(pv_next) <= s:
                        k, jb = divmod(pv_next, NJB)
                        _emit_pv(nc, cx, ots[k], blocks[k][0], jb,
                                 pts[pv_next])
                        if jb == NJB - 1:
                            _emit_norm(nc, cx, ots[k], *blocks[k],
                                       use_act=(k == NB - 1))
                        pv_next += 1
                # tail: out-projection for the last query tile; the g0/g1
                # halves depend only on the (0,3) norm, so they roll ahead
                # of the final norm's latency
                _emit_oproj_half(nc, cx, NT - 1, 0, 0)
                _emit_oproj_half(nc, cx, NT - 1, 1, 0)
                for gidx in range(8):
                    _emit_oproj_half(nc, cx, NT - 1, gidx, 1)
                    if gidx + 2 < 8:
                        _emit_oproj_half(nc, cx, NT - 1, gidx + 2, 0)

    nc.finalize()
    return nc


def round_f32r(a):
    """Round fp32 array to the fp32r grid (11 mantissa bits, RNE)."""
    u = np.ascontiguousarray(a, dtype=np.float32).view(np.uint32)
    u = (u + 0x7FF + ((u >> 12) & 1)) & np.uint32(0xFFFFF000)
    return u.view(np.float32)


def shard_inputs(x, w_qkv, w_out):
    """Full inputs -> list of 8 per-core input maps (host-side prep)."""
    x = np.asarray(x, dtype=np.float32)
    w_qkv = np.asarray(w_qkv, dtype=np.float32)
    w_out = np.asarray(w_out, dtype=np.float32)
    in_maps = []
    for c in range(8):
        b, hp = c // 4, c % 4
        rows = np.concatenate(
            [w_qkv[q * C + hp * HPC * D:(q * C + (hp + 1) * HPC * D)]
             for q in range(3)], axis=0)                      # [768, C]
        in_maps.append({
            "vones": np.ones((P, NJB * HPC), np.float32),
            "xT": np.ascontiguousarray(x[b].T).astype(
                ml_dtypes.bfloat16),                           # [C, N]
            "wqkvT": np.ascontiguousarray(
                rows.T.reshape(KB, P, 6, P).transpose(1, 2, 0, 3)).astype(
                ml_dtypes.bfloat16),                   # [128, 6, 8, 128]
            "woutT": np.ascontiguousarray(
                w_out[:, hp * HPC * D:(hp + 1) * HPC * D].T   # [256, C]
                .reshape(HPC, D, C)).astype(ml_dtypes.bfloat16),
        })
    return in_maps


def combine_outputs(ys, b_out):
    b_out = np.asarray(b_out, dtype=np.float32)
    ys = [np.asarray(v).astype(np.float32) for v in ys]
    out0 = ys[0] + ys[1] + ys[2] + ys[3]
    out1 = ys[4] + ys[5] + ys[6] + ys[7]
    return np.stack([out0, out1], axis=0) + b_out[None, None, :]


_NC = None


def kernel(x, w_qkv, w_out, b_out):
    global _NC
    if _NC is None:
        _NC = build_nc()
    in_maps = shard_inputs(x, w_qkv, w_out)
    res = run_bass_kernel_spmd(_NC, in_maps, core_ids=list(range(8)))
    ys = [res.results[c]["y"] for c in range(8)]
    return combine_outputs(ys, b_out).astype(np.float32)
